# revision 2
# baseline (speedup 1.0000x reference)
"""Trainium2 Bass kernel v2 for nn_CrossConvLayerV2 (gnn_message_passing).

Math (reference):
    coords = points[..., :3]; feats = points[..., 3:]          # [B,n,3], [B,n,f]
    probes[b,l,m] = centers[b,l] + PROBES[m]                    # [B,l,m,3]
    sq[b,l,m,n]  = ||coords[b,n] - probes[b,l,m]||^2
    kern         = C / (sq + C)          (C = 0.1)
    agg[b,l,m,f] = (1/n) sum_n kern * feats
    out[b,l,:]   = agg.reshape(l, m*f) @ W + bias               # [B,l,256]

v2 strategy (per core; centers sharded l/8 = 32 per core, 4 jobs of 16):
  - sq via ONE K=24 bf16 matmul per (job, chunk-of-128-points): the squared
    distance is expanded (cn2 - 2<c,p> + pn2) with every term split into
    three bf16 pieces (~24-bit exact).  The 4 jobs of a chunk run as a
    4-way ROW-TILED quad (tile_position=(32j,0), row group j) -> ~3x PE
    concurrency.  c5/p5 data is replicated per row-quadrant (quadrant j
    holds job j's batch).
  - sq pairs stream into alternating 4-bank / 3-bank PSUM tiles (double
    buffered ring over 7 banks).  kern = 1/u: one batched ACT Reciprocal
    per tile (the engine floor: ~1 elem/lane/cycle + 172cyc/inst).
    kt is fp16 in SBUF.
  - agg[f,(m l')] += ft_chunk^T @ kt: 4-way COL-TILED quad per chunk
    (tile_position=(0,32j), col group j) accumulating into one PSUM bank
    (quadrant j partitions hold job j's agg).
  - weighter: agg -> SBUF fp16, DMA-assembled into an (m%8,f)-partition
    packed layout, then 4 K=128 matmuls against host-packed W.  The 1/n
    mean, a 64x output scale (fp16-subnormal guard), and b_weighter are
    folded in on the host.
  - This walrus build encodes at most ONE semaphore wait per instruction;
    a post-build pass splits multi-wait instructions into single-wait
    NoOp carriers.
"""

import sys

sys.path.insert(0, "/opt/trn_rl_repo")

import numpy as np
import ml_dtypes

# ---- problem constants (hardcoded per contract) ----
B, N, L, D, F = 2, 4096, 256, 3, 16
M = 26
OUT_D = 256
COEFF = 0.1
DIST = 3.0
N_CORES = 8
L_LOC = L // N_CORES          # 32 centers per core
N_SLABS = 2
L_SLAB = L_LOC // N_SLABS     # 16 centers per job
JM = M * L_SLAB               # 416
N_JOBS = B * N_SLABS          # 4 jobs per core; job j: b=j//2, slab=j%2
NT = N // 128                 # 32 chunks
K5 = 24                       # expanded-distance contraction depth
NPAIR = NT * N_JOBS           # 128 (job, chunk) pairs; pair P: t=P//4, j=P%4
TPP = 3                       # pairs per sq PSUM tile (3 banks)
DVE_PAIR_EVERY = 10**9        # DVE reciprocal channel disabled (net-negative)
DVE_DELAY = 0                 # DVE-pair sq MMs not delayed (copy decouples)
SCALE_OUT = 64.0              # extra output scale (fp16 weight subnormal guard)


def _make_probes() -> np.ndarray:
    angles = np.array(
        [[j * 0.125 - 0.125, i * 0.125 + (j - 1) * 0.0625] for j in range(3) for i in range(8)]
        + [[-0.25, 0.0], [0.25, 0.0]],
        dtype=np.float64,
    ) * (2.0 * np.pi)
    a, b = angles[:, 0], angles[:, 1]
    pts = np.stack([np.sin(a), np.cos(a) * np.cos(b), np.cos(a) * np.sin(b)], axis=-1) * DIST
    return pts.astype(np.float32)  # [26, 3]


PROBES = _make_probes()


def _split3_bf16(x):
    """x (f64) -> three bf16 arrays whose sum approximates x to ~24 bits."""
    x0 = x.astype(ml_dtypes.bfloat16)
    r1 = x - x0.astype(np.float64)
    x1 = r1.astype(ml_dtypes.bfloat16)
    x2 = (r1 - x1.astype(np.float64)).astype(ml_dtypes.bfloat16)
    return x0, x1, x2


def _act_reciprocal(nc, out_ap, in_ap):
    """nc.scalar.activation(func=Reciprocal) minus the library guard."""
    import concourse.mybir as mybir

    eng = nc.scalar
    inputs = [eng.lower_ap(in_ap)]
    for val in (0.0, 1.0, 0.0):  # bias, scale, alpha — immediates
        inputs.append(mybir.ImmediateValue(dtype=mybir.dt.float32, value=val))
    return eng.add_instruction(
        mybir.InstActivation(
            name=nc.get_next_instruction_name(),
            func=mybir.ActivationFunctionType.Reciprocal,
            ins=inputs,
            outs=[eng.lower_ap(out_ap)],
        )
    )


def _split_multi_waits(nc):
    """Split every instruction with k>1 semaphore waits into (k-1)
    single-wait NoOps on the same engine immediately before it."""
    import concourse.mybir as mybir

    n = 0
    for f in nc.m.functions:
        for bb in f.blocks:
            new_il = []
            for inst in bb.instructions:
                si = inst.sync_info
                waits = list(si.on_wait) if si is not None else []
                if len(waits) > 1:
                    for w in waits[:-1]:
                        nop = mybir.InstNoOp(name=f"{inst.name}-wsplit{n}", ins=[], outs=[])
                        n += 1
                        nop.engine = inst.engine
                        nop.sync_info = mybir.SyncInfo(on_wait=[w], on_update=[])
                        nc.register_instruction(nop, overwrite=True)
                        new_il.append(nop)
                    inst.sync_info = mybir.SyncInfo(
                        on_wait=[waits[-1]], on_update=list(si.on_update)
                    )
                new_il.append(inst)
            bb.instructions = new_il
    return n


_NC = None


def _build_nc():
    import concourse.bass as bass
    import concourse.mybir as mybir
    import concourse.tile as tile

    f32 = mybir.dt.float32
    bf16 = mybir.dt.bfloat16
    fp16 = mybir.dt.float16

    nc = bass.Bass()
    # quadrant-replicated inputs: rows 32j+k hold job j's data row k
    c5q_d = nc.dram_tensor("c5q", [128, N], bf16, kind="ExternalInput")
    p5q_d = nc.dram_tensor("p5q", [128, JM], bf16, kind="ExternalInput")
    ft_d = nc.dram_tensor("ft", [128, B * NT * F], fp16, kind="ExternalInput")
    wp_d = nc.dram_tensor("wp", [16, M * OUT_D], fp16, kind="ExternalInput")
    out_d = nc.dram_tensor("out", [N_JOBS * L_SLAB, OUT_D], f32, kind="ExternalOutput")

    with (
        nc.allow_low_precision(reason="split-bf16 matmul is ~24-bit exact"),
        tile.TileContext(nc) as tc,
    ):
        with (
            tc.tile_pool(name="const", bufs=1) as cpool,
            tc.tile_pool(name="kt", bufs=5) as ktpool,
            tc.tile_pool(name="sq", bufs=1, space="PSUM") as sqpool,
            tc.tile_pool(name="agg", bufs=1, space="PSUM") as aggpool,
        ):
            c5qs = cpool.tile([128, N], bf16)
            p5qs = cpool.tile([128, JM], bf16)
            fts = cpool.tile([128, B * NT * F], fp16)
            wps = cpool.tile([16, M * OUT_D], fp16)
            # input DMAs off the gpsimd queue (25ns dispatch vs 565 on sync),
            # c5q split so compute can start after the first piece
            nc.sync.dma_start(p5qs[0:64, :], p5q_d[0:64, :])
            nc.scalar.dma_start(p5qs[64:128, :], p5q_d[64:128, :])
            nc.gpsimd.dma_start(c5qs[:, 0:128], c5q_d[:, 0:128])
            nc.scalar.dma_start(fts[:], ft_d[:, :])
            nc.gpsimd.dma_start(c5qs[:, 128:1280], c5q_d[:, 128:1280])
            nc.sync.dma_start(c5qs[:, 1280:2560], c5q_d[:, 1280:2560])
            nc.scalar.dma_start(c5qs[:, 2560:4096], c5q_d[:, 2560:4096])
            nc.sync.dma_start(wps[:], wp_d[:, :])

            aggP = aggpool.tile([128, JM], f32)

            # pair -> kt access-pattern slice (filled as recips are emitted)
            kt_ap = {}
            agg_done = [0]

            def emit_agg(t):
                for j in range(N_JOBS):
                    P = 4 * t + j
                    b = j // 2
                    nc.tensor.matmul(
                        aggP[32 * j : 32 * j + 16, :],
                        lhsT=fts[:, (b * NT + t) * F : (b * NT + t + 1) * F],
                        rhs=kt_ap[P],
                        start=(t == 0),
                        stop=(t == NT - 1),
                        tile_position=(0, 32 * j),
                    )

            def emit_sq_mm(dst_ap, P):
                t, j = P // 4, P % 4
                nc.tensor.matmul(
                    dst_ap,
                    lhsT=c5qs[32 * j : 32 * j + K5, t * 128 : (t + 1) * 128],
                    rhs=p5qs[32 * j : 32 * j + K5, :],
                    start=True,
                    stop=True,
                    tile_position=(32 * j, 0),
                )

            # stream pairs: every DVE_PAIR_EVERY-th goes through the 1-bank
            # DVE side channel; the rest batch 3-per-tile through ACT
            act_stream = []
            tag_a = True  # alternate 4-bank (A) and 3-bank (B) sq tiles
            for P in range(NPAIR):
                act_stream.append(P)
                cap = 4 if tag_a else 3
                if len(act_stream) == cap or P == NPAIR - 1:
                    npp = len(act_stream)
                    Tn = act_stream[0]
                    if tag_a:
                        sq = sqpool.tile([128, 4 * 512], f32, tag="sqA", name=f"sq{Tn}")
                    else:
                        sq = sqpool.tile([128, 3 * 512], f32, tag="sqB", name=f"sq{Tn}")
                    for i, Q in enumerate(act_stream):
                        emit_sq_mm(sq[:, i * 512 : i * 512 + 416], Q)
                    kt = ktpool.tile([128, 4 * 416], fp16, tag="kt", name=f"kt{Tn}")
                    sqv = sq[:].rearrange("p (u x) -> p u x", u=cap)[:, 0:npp, 0:416]
                    ktv = kt[:].rearrange("p (u x) -> p u x", u=4)[:, 0:npp, 0:416]
                    _act_reciprocal(nc, ktv, sqv)
                    for i, Q in enumerate(act_stream):
                        kt_ap[Q] = kt[:, i * 416 : (i + 1) * 416]
                    act_stream = []
                    tag_a = not tag_a
                # emit agg lagging ~1 chunk behind the pair stream
                while (agg_done[0] + 1) * 4 <= P - 3:
                    emit_agg(agg_done[0])
                    agg_done[0] += 1
            while agg_done[0] < NT:
                emit_agg(agg_done[0])
                agg_done[0] += 1

            # ---- weighter ----
            aggSs = cpool.tile([128, JM], fp16)
            nc.vector.tensor_copy(aggSs[:], aggP[:])

            # stage[f, (m, j, l')] <- aggS[32j+f, (m, l')]: one DMA per job
            stage = cpool.tile([16, M * 64], fp16)
            qs2 = [nc.gpsimd, nc.scalar, nc.sync, nc.gpsimd]
            for j in range(N_JOBS):
                srcq = aggSs[32 * j : 32 * j + 16, :]
                dst = stage[:].rearrange("p (m j l) -> p m j l", m=M, j=4)[:, :, j, :]
                qs2[j].dma_start(dst, srcq.rearrange("p (m l) -> p m l", m=M))

            op = sqpool.tile([N_JOBS * L_SLAB, OUT_D], f32, tag="sqB", name="op")
            for m in range(M):
                nc.tensor.matmul(
                    op[:],
                    lhsT=stage[:, m * 64 : (m + 1) * 64],
                    rhs=wps[:, m * OUT_D : (m + 1) * OUT_D],
                    start=(m == 0),
                    stop=(m == M - 1),
                )
            oS = cpool.tile([N_JOBS * L_SLAB, OUT_D], f32)
            nc.vector.tensor_copy(oS[:], op[:])
            nc.sync.dma_start(out_d[0:32, :], oS[0:32, :])
            nc.gpsimd.dma_start(out_d[32:64, :], oS[32:64, :])

    _split_multi_waits(nc)
    return nc


def _get_nc():
    global _NC
    if _NC is None:
        _NC = _build_nc()
    return _NC


def _prep_shared(points, W_weighter):
    coords = points[:, :, :D].astype(np.float64)           # [B, n, 3]
    feats = points[:, :, D:].astype(np.float32)            # [B, n, f]
    q = 10.0 * (coords**2).sum(-1)                         # [B, n] f64

    c5 = np.zeros((K5, B * N), ml_dtypes.bfloat16)
    for b in range(B):
        s = slice(b * N, (b + 1) * N)
        for k in range(D):
            c0, c1, c2 = _split3_bf16(coords[b, :, k])
            base = 6 * k
            c5[base + 0, s] = c0
            c5[base + 1, s] = c0
            c5[base + 2, s] = c1
            c5[base + 3, s] = c1
            c5[base + 4, s] = c2
            c5[base + 5, s] = c0
        c5[18:21, s] = 1.0
        q0, q1, q2 = _split3_bf16(q[b])
        c5[21, s] = q0
        c5[22, s] = q1
        c5[23, s] = q2

    # quadrant-replicated c5: quadrant j holds c5 of batch j//2
    c5q = np.zeros((128, N), ml_dtypes.bfloat16)
    for j in range(N_JOBS):
        b = j // 2
        c5q[32 * j : 32 * j + K5, :] = c5[:, b * N : (b + 1) * N]

    # ft[p, (b, t, f)] = feats[b, t*128+p, f]   (fp16)
    ft = (
        np.ascontiguousarray(feats.reshape(B, NT, 128, F).transpose(2, 0, 1, 3))
        .reshape(128, B * NT * F)
        .astype(np.float16)
    )

    # wp[f, m*256+o] = W[m*F+f, o] * (8*SCALE_OUT/N)
    w = W_weighter.astype(np.float64).reshape(M, F, OUT_D) * (8.0 * SCALE_OUT / N)
    wp = np.ascontiguousarray(w.transpose(1, 0, 2)).reshape(16, M * OUT_D).astype(np.float16)
    return c5q, ft, wp


def _prep_p5q(centers, core):
    cen = centers[:, core * L_LOC : (core + 1) * L_LOC, :].astype(np.float64)  # [B, 32, 3]
    p5q = np.zeros((128, JM), ml_dtypes.bfloat16)
    for j in range(N_JOBS):
        b, sl_i = j // 2, j % 2
        sl = cen[b, sl_i * L_SLAB : (sl_i + 1) * L_SLAB]       # [16, 3]
        pf = sl[:, None, :] + PROBES[None].astype(np.float64)  # [16, 26, 3]
        mlf = pf.transpose(1, 0, 2).reshape(JM, 3)             # (m, l') major
        base_row = 32 * j
        for k in range(D):
            p0, p1, p2 = _split3_bf16(8.0 * -20.0 * mlf[:, k])
            base = base_row + 6 * k
            p5q[base + 0, :] = p0
            p5q[base + 1, :] = p1
            p5q[base + 2, :] = p0
            p5q[base + 3, :] = p1
            p5q[base + 4, :] = p0
            p5q[base + 5, :] = p2
        r = 8.0 * (10.0 * (mlf**2).sum(-1) + 1.0)              # [JM] f64
        r0, r1, r2 = _split3_bf16(r)
        p5q[base_row + 18, :] = r0
        p5q[base_row + 19, :] = r1
        p5q[base_row + 20, :] = r2
        p5q[base_row + 21 : base_row + 24, :] = 8.0
    return p5q


def kernel(points, centers, W_weighter, b_weighter):
    from concourse.bass_utils import run_bass_kernel_spmd

    points = np.asarray(points)
    centers = np.asarray(centers)
    W_weighter = np.asarray(W_weighter)
    b_weighter = np.asarray(b_weighter)

    nc = _get_nc()
    c5q, ft, wp = _prep_shared(points, W_weighter)
    in_maps = [
        {"c5q": c5q, "ft": ft, "p5q": _prep_p5q(centers, core), "wp": wp}
        for core in range(N_CORES)
    ]
    res = run_bass_kernel_spmd(nc, in_maps, core_ids=list(range(N_CORES))).results

    out = np.empty((B, L, OUT_D), np.float32)
    for core in range(N_CORES):
        r = res[core]["out"]  # [(j, l'), OUT_D]
        for j in range(N_JOBS):
            b, s = j // 2, j % 2
            lo = core * L_LOC + s * L_SLAB
            out[b, lo : lo + L_SLAB] = r[j * L_SLAB : (j + 1) * L_SLAB]
    out *= 1.0 / SCALE_OUT
    out += b_weighter.astype(np.float32)[None, None, :]
    return out


# revision 3
# speedup vs baseline: 1.0119x; 1.0119x over previous
"""Trainium2 Bass kernel v2 for nn_CrossConvLayerV2 (gnn_message_passing).

Math (reference):
    coords = points[..., :3]; feats = points[..., 3:]          # [B,n,3], [B,n,f]
    probes[b,l,m] = centers[b,l] + PROBES[m]                    # [B,l,m,3]
    sq[b,l,m,n]  = ||coords[b,n] - probes[b,l,m]||^2
    kern         = C / (sq + C)          (C = 0.1)
    agg[b,l,m,f] = (1/n) sum_n kern * feats
    out[b,l,:]   = agg.reshape(l, m*f) @ W + bias               # [B,l,256]

v2 strategy (per core; centers sharded l/8 = 32 per core, 4 jobs of 16):
  - sq via ONE K=24 bf16 matmul per (job, chunk-of-128-points): the squared
    distance is expanded (cn2 - 2<c,p> + pn2) with every term split into
    three bf16 pieces (~24-bit exact).  The 4 jobs of a chunk run as a
    4-way ROW-TILED quad (tile_position=(32j,0), row group j) -> ~3x PE
    concurrency.  c5/p5 data is replicated per row-quadrant (quadrant j
    holds job j's batch).
  - sq pairs stream into alternating 4-bank / 3-bank PSUM tiles (double
    buffered ring over 7 banks).  kern = 1/u: one batched ACT Reciprocal
    per tile (the engine floor: ~1 elem/lane/cycle + 172cyc/inst).
    kt is fp16 in SBUF.
  - agg[f,(m l')] += ft_chunk^T @ kt: 4-way COL-TILED quad per chunk
    (tile_position=(0,32j), col group j) accumulating into one PSUM bank
    (quadrant j partitions hold job j's agg).
  - weighter: agg -> SBUF fp16, DMA-assembled into an (m%8,f)-partition
    packed layout, then 4 K=128 matmuls against host-packed W.  The 1/n
    mean, a 64x output scale (fp16-subnormal guard), and b_weighter are
    folded in on the host.
  - This walrus build encodes at most ONE semaphore wait per instruction;
    a post-build pass splits multi-wait instructions into single-wait
    NoOp carriers.
"""

import sys

sys.path.insert(0, "/opt/trn_rl_repo")

import numpy as np
import ml_dtypes

# ---- problem constants (hardcoded per contract) ----
B, N, L, D, F = 2, 4096, 256, 3, 16
M = 26
OUT_D = 256
COEFF = 0.1
DIST = 3.0
N_CORES = 8
L_LOC = L // N_CORES          # 32 centers per core
N_SLABS = 2
L_SLAB = L_LOC // N_SLABS     # 16 centers per job
JM = M * L_SLAB               # 416
N_JOBS = B * N_SLABS          # 4 jobs per core; job j: b=j//2, slab=j%2
NT = N // 128                 # 32 chunks
K5 = 24                       # expanded-distance contraction depth
NPAIR = NT * N_JOBS           # 128 (job, chunk) pairs; pair P: t=P//4, j=P%4
TPP = 3                       # pairs per sq PSUM tile (3 banks)
DVE_PAIR_EVERY = 10**9        # DVE reciprocal channel disabled (net-negative)
DVE_DELAY = 0                 # DVE-pair sq MMs not delayed (copy decouples)
SCALE_OUT = 64.0              # extra output scale (fp16 weight subnormal guard)


def _make_probes() -> np.ndarray:
    angles = np.array(
        [[j * 0.125 - 0.125, i * 0.125 + (j - 1) * 0.0625] for j in range(3) for i in range(8)]
        + [[-0.25, 0.0], [0.25, 0.0]],
        dtype=np.float64,
    ) * (2.0 * np.pi)
    a, b = angles[:, 0], angles[:, 1]
    pts = np.stack([np.sin(a), np.cos(a) * np.cos(b), np.cos(a) * np.sin(b)], axis=-1) * DIST
    return pts.astype(np.float32)  # [26, 3]


PROBES = _make_probes()


def _split3_bf16(x):
    """x (f64) -> three bf16 arrays whose sum approximates x to ~24 bits."""
    x0 = x.astype(ml_dtypes.bfloat16)
    r1 = x - x0.astype(np.float64)
    x1 = r1.astype(ml_dtypes.bfloat16)
    x2 = (r1 - x1.astype(np.float64)).astype(ml_dtypes.bfloat16)
    return x0, x1, x2


def _act_reciprocal(nc, out_ap, in_ap):
    """nc.scalar.activation(func=Reciprocal) minus the library guard."""
    import concourse.mybir as mybir

    eng = nc.scalar
    inputs = [eng.lower_ap(in_ap)]
    for val in (0.0, 1.0, 0.0):  # bias, scale, alpha — immediates
        inputs.append(mybir.ImmediateValue(dtype=mybir.dt.float32, value=val))
    return eng.add_instruction(
        mybir.InstActivation(
            name=nc.get_next_instruction_name(),
            func=mybir.ActivationFunctionType.Reciprocal,
            ins=inputs,
            outs=[eng.lower_ap(out_ap)],
        )
    )


def _split_multi_waits(nc):
    """Split every instruction with k>1 semaphore waits into (k-1)
    single-wait NoOps on the same engine immediately before it."""
    import concourse.mybir as mybir

    n = 0
    for f in nc.m.functions:
        for bb in f.blocks:
            new_il = []
            for inst in bb.instructions:
                si = inst.sync_info
                waits = list(si.on_wait) if si is not None else []
                if len(waits) > 1:
                    for w in waits[:-1]:
                        nop = mybir.InstNoOp(name=f"{inst.name}-wsplit{n}", ins=[], outs=[])
                        n += 1
                        nop.engine = inst.engine
                        nop.sync_info = mybir.SyncInfo(on_wait=[w], on_update=[])
                        nc.register_instruction(nop, overwrite=True)
                        new_il.append(nop)
                    inst.sync_info = mybir.SyncInfo(
                        on_wait=[waits[-1]], on_update=list(si.on_update)
                    )
                new_il.append(inst)
            bb.instructions = new_il
    return n


_NC = None


def _build_nc():
    import concourse.bass as bass
    import concourse.mybir as mybir
    import concourse.tile as tile

    f32 = mybir.dt.float32
    bf16 = mybir.dt.bfloat16
    fp16 = mybir.dt.float16

    nc = bass.Bass()
    # quadrant-replicated inputs: rows 32j+k hold job j's data row k
    c5q_d = nc.dram_tensor("c5q", [128, N], bf16, kind="ExternalInput")
    p5q_d = nc.dram_tensor("p5q", [128, JM], bf16, kind="ExternalInput")
    ft_d = nc.dram_tensor("ft", [128, B * NT * F], fp16, kind="ExternalInput")
    wp_d = nc.dram_tensor("wp", [16, M * OUT_D], fp16, kind="ExternalInput")
    out_d = nc.dram_tensor("out", [N_JOBS * L_SLAB, OUT_D], f32, kind="ExternalOutput")

    with (
        nc.allow_low_precision(reason="split-bf16 matmul is ~24-bit exact"),
        tile.TileContext(nc) as tc,
    ):
        with (
            tc.tile_pool(name="const", bufs=1) as cpool,
            tc.tile_pool(name="kt", bufs=5) as ktpool,
            tc.tile_pool(name="sq", bufs=1, space="PSUM") as sqpool,
            tc.tile_pool(name="agg", bufs=1, space="PSUM") as aggpool,
        ):
            c5qs = cpool.tile([128, N], bf16)
            p5qs = cpool.tile([128, JM], bf16)
            fts = cpool.tile([128, B * NT * F], fp16)
            wps = cpool.tile([16, M * OUT_D], fp16)
            # input DMAs off the gpsimd queue (25ns dispatch vs 565 on sync),
            # c5q split so compute can start after the first piece
            nc.sync.dma_start(p5qs[0:64, :], p5q_d[0:64, :])
            nc.scalar.dma_start(p5qs[64:128, :], p5q_d[64:128, :])
            nc.gpsimd.dma_start(c5qs[:, 0:128], c5q_d[:, 0:128])
            nc.scalar.dma_start(fts[:], ft_d[:, :])
            nc.gpsimd.dma_start(c5qs[:, 128:1280], c5q_d[:, 128:1280])
            nc.sync.dma_start(c5qs[:, 1280:2560], c5q_d[:, 1280:2560])
            nc.scalar.dma_start(c5qs[:, 2560:4096], c5q_d[:, 2560:4096])
            nc.sync.dma_start(wps[:], wp_d[:, :])

            aggP = aggpool.tile([128, JM], f32)

            # pair -> kt access-pattern slice (filled as recips are emitted)
            kt_ap = {}
            agg_done = [0]

            def emit_agg(t):
                for j in range(N_JOBS):
                    P = 4 * t + j
                    b = j // 2
                    nc.tensor.matmul(
                        aggP[32 * j : 32 * j + 16, :],
                        lhsT=fts[:, (b * NT + t) * F : (b * NT + t + 1) * F],
                        rhs=kt_ap[P],
                        start=(t == 0),
                        stop=(t == NT - 1),
                        tile_position=(0, 32 * j),
                    )

            def emit_sq_mm(dst_ap, P):
                t, j = P // 4, P % 4
                nc.tensor.matmul(
                    dst_ap,
                    lhsT=c5qs[32 * j : 32 * j + K5, t * 128 : (t + 1) * 128],
                    rhs=p5qs[32 * j : 32 * j + K5, :],
                    start=True,
                    stop=True,
                    tile_position=(32 * j, 0),
                )

            # stream pairs: every DVE_PAIR_EVERY-th goes through the 1-bank
            # DVE side channel; the rest batch 3-per-tile through ACT
            act_stream = []
            tag_a = True  # alternate 4-bank (A) and 3-bank (B) sq tiles
            for P in range(NPAIR):
                act_stream.append(P)
                cap = 4 if tag_a else 3
                if len(act_stream) == cap or P == NPAIR - 1:
                    npp = len(act_stream)
                    Tn = act_stream[0]
                    if tag_a:
                        sq = sqpool.tile([128, 4 * 512], f32, tag="sqA", name=f"sq{Tn}")
                    else:
                        sq = sqpool.tile([128, 3 * 512], f32, tag="sqB", name=f"sq{Tn}")
                    for i, Q in enumerate(act_stream):
                        emit_sq_mm(sq[:, i * 512 : i * 512 + 416], Q)
                    kt = ktpool.tile([128, 4 * 416], fp16, tag="kt", name=f"kt{Tn}")
                    sqv = sq[:].rearrange("p (u x) -> p u x", u=cap)[:, 0:npp, 0:416]
                    ktv = kt[:].rearrange("p (u x) -> p u x", u=4)[:, 0:npp, 0:416]
                    _act_reciprocal(nc, ktv, sqv)
                    for i, Q in enumerate(act_stream):
                        kt_ap[Q] = kt[:, i * 416 : (i + 1) * 416]
                    act_stream = []
                    tag_a = not tag_a
                # emit agg lagging ~1 chunk behind the pair stream
                while (agg_done[0] + 1) * 4 <= P - 3:
                    emit_agg(agg_done[0])
                    agg_done[0] += 1
            while agg_done[0] < NT:
                emit_agg(agg_done[0])
                agg_done[0] += 1

            # ---- weighter ----
            aggSs = cpool.tile([128, JM], fp16)
            nc.vector.tensor_copy(aggSs[:], aggP[:])

            # stage[f, (m, j, l')] <- aggS[32j+f, (m, l')]: one DMA per job
            stage = cpool.tile([16, M * 64], fp16)
            qs2 = [nc.gpsimd, nc.scalar, nc.sync, nc.scalar]
            for j in range(N_JOBS):
                srcq = aggSs[32 * j : 32 * j + 16, :]
                dst = stage[:].rearrange("p (m j l) -> p m j l", m=M, j=4)[:, :, j, :]
                qs2[j].dma_start(dst, srcq.rearrange("p (m l) -> p m l", m=M))

            op = sqpool.tile([N_JOBS * L_SLAB, OUT_D], f32, tag="sqB", name="op")
            for m in range(M):
                nc.tensor.matmul(
                    op[:],
                    lhsT=stage[:, m * 64 : (m + 1) * 64],
                    rhs=wps[:, m * OUT_D : (m + 1) * OUT_D],
                    start=(m == 0),
                    stop=(m == M - 1),
                )
            oS = cpool.tile([N_JOBS * L_SLAB, OUT_D], f32)
            nc.vector.tensor_copy(oS[0:32, :], op[0:32, :])
            nc.sync.dma_start(out_d[0:32, :], oS[0:32, :])
            nc.vector.tensor_copy(oS[32:64, :], op[32:64, :])
            nc.gpsimd.dma_start(out_d[32:64, :], oS[32:64, :])

    _split_multi_waits(nc)
    return nc


def _get_nc():
    global _NC
    if _NC is None:
        _NC = _build_nc()
    return _NC


def _prep_shared(points, W_weighter):
    coords = points[:, :, :D].astype(np.float64)           # [B, n, 3]
    feats = points[:, :, D:].astype(np.float32)            # [B, n, f]
    q = 10.0 * (coords**2).sum(-1)                         # [B, n] f64

    c5 = np.zeros((K5, B * N), ml_dtypes.bfloat16)
    for b in range(B):
        s = slice(b * N, (b + 1) * N)
        for k in range(D):
            c0, c1, c2 = _split3_bf16(coords[b, :, k])
            base = 6 * k
            c5[base + 0, s] = c0
            c5[base + 1, s] = c0
            c5[base + 2, s] = c1
            c5[base + 3, s] = c1
            c5[base + 4, s] = c2
            c5[base + 5, s] = c0
        c5[18:21, s] = 1.0
        q0, q1, q2 = _split3_bf16(q[b])
        c5[21, s] = q0
        c5[22, s] = q1
        c5[23, s] = q2

    # quadrant-replicated c5: quadrant j holds c5 of batch j//2
    c5q = np.zeros((128, N), ml_dtypes.bfloat16)
    for j in range(N_JOBS):
        b = j // 2
        c5q[32 * j : 32 * j + K5, :] = c5[:, b * N : (b + 1) * N]

    # ft[p, (b, t, f)] = feats[b, t*128+p, f]   (fp16)
    ft = (
        np.ascontiguousarray(feats.reshape(B, NT, 128, F).transpose(2, 0, 1, 3))
        .reshape(128, B * NT * F)
        .astype(np.float16)
    )

    # wp[f, m*256+o] = W[m*F+f, o] * (8*SCALE_OUT/N)
    w = W_weighter.astype(np.float64).reshape(M, F, OUT_D) * (8.0 * SCALE_OUT / N)
    wp = np.ascontiguousarray(w.transpose(1, 0, 2)).reshape(16, M * OUT_D).astype(np.float16)
    return c5q, ft, wp


def _prep_p5q(centers, core):
    cen = centers[:, core * L_LOC : (core + 1) * L_LOC, :].astype(np.float64)  # [B, 32, 3]
    p5q = np.zeros((128, JM), ml_dtypes.bfloat16)
    for j in range(N_JOBS):
        b, sl_i = j // 2, j % 2
        sl = cen[b, sl_i * L_SLAB : (sl_i + 1) * L_SLAB]       # [16, 3]
        pf = sl[:, None, :] + PROBES[None].astype(np.float64)  # [16, 26, 3]
        mlf = pf.transpose(1, 0, 2).reshape(JM, 3)             # (m, l') major
        base_row = 32 * j
        for k in range(D):
            p0, p1, p2 = _split3_bf16(8.0 * -20.0 * mlf[:, k])
            base = base_row + 6 * k
            p5q[base + 0, :] = p0
            p5q[base + 1, :] = p1
            p5q[base + 2, :] = p0
            p5q[base + 3, :] = p1
            p5q[base + 4, :] = p0
            p5q[base + 5, :] = p2
        r = 8.0 * (10.0 * (mlf**2).sum(-1) + 1.0)              # [JM] f64
        r0, r1, r2 = _split3_bf16(r)
        p5q[base_row + 18, :] = r0
        p5q[base_row + 19, :] = r1
        p5q[base_row + 20, :] = r2
        p5q[base_row + 21 : base_row + 24, :] = 8.0
    return p5q


def kernel(points, centers, W_weighter, b_weighter):
    from concourse.bass_utils import run_bass_kernel_spmd

    points = np.asarray(points)
    centers = np.asarray(centers)
    W_weighter = np.asarray(W_weighter)
    b_weighter = np.asarray(b_weighter)

    nc = _get_nc()
    c5q, ft, wp = _prep_shared(points, W_weighter)
    in_maps = [
        {"c5q": c5q, "ft": ft, "p5q": _prep_p5q(centers, core), "wp": wp}
        for core in range(N_CORES)
    ]
    res = run_bass_kernel_spmd(nc, in_maps, core_ids=list(range(N_CORES))).results

    out = np.empty((B, L, OUT_D), np.float32)
    for core in range(N_CORES):
        r = res[core]["out"]  # [(j, l'), OUT_D]
        for j in range(N_JOBS):
            b, s = j // 2, j % 2
            lo = core * L_LOC + s * L_SLAB
            out[b, lo : lo + L_SLAB] = r[j * L_SLAB : (j + 1) * L_SLAB]
    out *= 1.0 / SCALE_OUT
    out += b_weighter.astype(np.float32)[None, None, :]
    return out
